# revision 5
# baseline (speedup 1.0000x reference)
"""Trainium2 Bass kernel for the hard-negative-mining set loss (v5).

Structure:
  * host: positives (first/second same-class occurrence) and per-class
    member tables are pure index bookkeeping on `target`; computed in
    numpy and shipped as gather tables (like the baseline's index tables).
  * device mining (row-sharded): per-class hardest negative via a packed
    f32 key K = 8192*q + enc - 2^24*same_mask, q = quantized -log prob,
    enc = 8191 - global_row. Quantization uses the fp32 "magic constant"
    rounding trick (adding 1.5*2^36 rounds to a multiple of 8192).
    partition_all_reduce collapses local rows; a 4KB ReduceScatter(max)
    yields each core's 128 owned classes' global keys directly.
  * device CE (class-sharded): per class only 7 distinct summed rows
    (anchors i0/i1 share one), members prefetched from t=0 via host
    tables; only the 128 neg rows (512KB) wait on the collective.
"""

import numpy as np

import concourse.bass as bass
import concourse.bacc as bacc
import concourse.tile as tile
from concourse import mybir
from concourse import bass_isa
from concourse.bass_utils import run_bass_kernel_spmd

B, C = 8192, 1024
NCORES = 8
BL = B // NCORES      # 1024 local mining rows per core
NT = BL // 128        # 8 row tiles
CCL = C // NCORES     # 128 classes owned per core
M = B // C            # 8 members per class

SHIFT_A = 10.0        # mining softmax shift
SHIFT_C = 14.0        # summed-logits softmax shift
QSCALE = 140.0        # log-prob quantization: 1/140 nat resolution
SCALE = QSCALE * 8192.0            # 1146880.0
M2 = 1.5 * (2.0 ** 36)             # magic: ulp(M2) = 8192
M2C = M2 + SCALE * SHIFT_A         # exact multiple of 8192
MASKC = -16777216.0                # -2^24 same-class exclusion
F32 = mybir.dt.float32
I32 = mybir.dt.int32
OP = mybir.AluOpType
AF = mybir.ActivationFunctionType


def build_nc():
    nc = bacc.Bacc("TRN2", target_bir_lowering=False, debug=False,
                   num_devices=NCORES)

    x_d = nc.dram_tensor("x", [B, C], F32, kind="ExternalInput")
    xloc_d = nc.dram_tensor("xloc", [BL, C], F32, kind="ExternalInput")
    xmem_d = nc.dram_tensor("xmem", [M * CCL, C], F32, kind="ExternalInput")
    cidb_d = nc.dram_tensor("cidb", [128, C], F32, kind="ExternalInput")
    tcols_d = nc.dram_tensor("tcols", [128, NT], F32, kind="ExternalInput")
    encs_d = nc.dram_tensor("encs", [128, NT], F32, kind="ExternalInput")
    eqmc_d = nc.dram_tensor("eqmc", [128, C], F32, kind="ExternalInput")
    dxm_d = nc.dram_tensor("dxm", [128, M], F32, kind="ExternalInput")
    w8_d = nc.dram_tensor("w8", [128, M], F32, kind="ExternalInput")
    w7_d = nc.dram_tensor("w7", [128, M - 1], F32, kind="ExternalInput")
    out_d = nc.dram_tensor("partial", [1, 1], F32, kind="ExternalOutput")

    cc_in = nc.dram_tensor("cc_in", [1, C], F32)
    cc_out = nc.dram_tensor("cc_out", [CCL, 1], F32)

    with tile.TileContext(nc) as tc:
        with (
            tc.tile_pool(name="persist", bufs=1) as pp,
            tc.tile_pool(name="rscr", bufs=3) as rp,
            tc.tile_pool(name="kscr", bufs=3) as kp,
            tc.tile_pool(name="mscr", bufs=3) as mp,
            tc.tile_pool(name="dumpC", bufs=2) as dcp,
            tc.tile_pool(name="small", bufs=6) as smp,
            tc.tile_pool(name="psB", bufs=1, space="PSUM") as psb,
        ):
            # ---------- input DMAs: xloc first (gates mining) ----------
            xloc = []
            for t in range(NT):
                xt = pp.tile([128, C], F32, tag=f"xloc{t}")
                nc.sync.dma_start(out=xt, in_=xloc_d.ap()[t * 128:(t + 1) * 128, :])
                xloc.append(xt)
            cidb = pp.tile([128, C], F32, tag="cidb")
            nc.sync.dma_start(out=cidb, in_=cidb_d.ap())
            tcols = pp.tile([128, NT], F32, tag="tcols")
            nc.sync.dma_start(out=tcols, in_=tcols_d.ap())
            encs = pp.tile([128, NT], F32, tag="encs")
            nc.sync.dma_start(out=encs, in_=encs_d.ap())
            dxm = pp.tile([128, M], F32, tag="dxm")
            nc.sync.dma_start(out=dxm, in_=dxm_d.ap())
            w8 = pp.tile([128, M], F32, tag="w8")
            nc.sync.dma_start(out=w8, in_=w8_d.ap())
            w7 = pp.tile([128, M - 1], F32, tag="w7")
            nc.sync.dma_start(out=w7, in_=w7_d.ap())
            xmem = []
            for m in range(M):
                xt = pp.tile([128, C], F32, tag=f"xmem{m}")
                nc.sync.dma_start(out=xt, in_=xmem_d.ap()[m * 128:(m + 1) * 128, :])
                xmem.append(xt)
            eqmc = pp.tile([128, C], F32, tag="eqmc")
            nc.sync.dma_start(out=eqmc, in_=eqmc_d.ap())

            ones = pp.tile([128, 1], F32, tag="ones")
            nc.vector.memset(ones, 1.0)
            shA = pp.tile([128, 1], F32, tag="shA")
            nc.vector.memset(shA, -SHIFT_A)
            shC = pp.tile([128, 1], F32, tag="shC")
            nc.vector.memset(shC, -SHIFT_C)

            # ---------- mining: packed-key build ----------
            # ACT queue: all Exp, all Ln, all Relu (one table load each)
            dumpA = pp.tile([128, C], F32, tag="dumpA")
            rs = []
            for t in range(NT):
                r = smp.tile([128, 1], F32, tag=f"rs{t}")
                nc.scalar.activation(out=dumpA, in_=xloc[t], func=AF.Exp,
                                     bias=shA, scale=1.0, accum_out=r)
                rs.append(r)
            lr = []
            for t in range(NT):
                l = smp.tile([128, 1], F32, tag=f"lr{t}")
                nc.scalar.activation(out=l, in_=rs[t], func=AF.Ln)
                lr.append(l)
            # b_t = f32(SCALE*lr + M2C): multiple of 8192 (carries lnrsum)
            bt = []
            for t in range(NT):
                b = smp.tile([128, 1], F32, tag=f"bt{t}")
                nc.vector.tensor_scalar(out=b, in0=lr[t], scalar1=SCALE,
                                        scalar2=M2C, op0=OP.mult, op1=OP.add)
                bt.append(b)
            # masks early on Pool (overlaps ACT work)
            masks = []
            for t in range(NT):
                mk = mp.tile([128, C], F32, tag="mask")
                nc.gpsimd.tensor_scalar(out=mk, in0=cidb,
                                        scalar1=tcols[:, t:t + 1],
                                        scalar2=MASKC,
                                        op0=OP.is_equal, op1=OP.mult)
                masks.append(mk)
            # r_t = Relu(-SCALE*x + b_t) = b_t + 8192*q (fp32 rounds @8192)
            Kacc = pp.tile([128, C], F32, tag="Kacc")
            for t in range(NT):
                rt = rp.tile([128, C], F32, tag="relu")
                nc.scalar.activation(out=rt, in_=xloc[t], func=AF.Relu,
                                     bias=bt[t], scale=-SCALE)
                q8e = kp.tile([128, C], F32, tag="q8e")
                nc.vector.tensor_scalar(out=q8e, in0=rt, scalar1=bt[t],
                                        scalar2=encs[:, t:t + 1],
                                        op0=OP.subtract, op1=OP.add)
                if t == 0:
                    nc.vector.tensor_tensor(out=Kacc, in0=q8e, in1=masks[t],
                                            op=OP.add)
                else:
                    kt = kp.tile([128, C], F32, tag="kt")
                    eng = nc.gpsimd if t in (1, 2, 4, 6) else nc.vector
                    eng.tensor_tensor(out=kt, in0=q8e, in1=masks[t],
                                      op=OP.add)
                    nc.vector.tensor_tensor(out=Kacc, in0=Kacc, in1=kt,
                                            op=OP.max)

            # local per-class max across 128 partitions, then 4KB RS(max)
            Kpar = pp.tile([128, C], F32, tag="Kpar")
            nc.gpsimd.partition_all_reduce(out_ap=Kpar, in_ap=Kacc,
                                           channels=128,
                                           reduce_op=bass_isa.ReduceOp.max)
            nc.gpsimd.dma_start(out=cc_in.ap(), in_=Kpar[0:1, :])
            nc.gpsimd.collective_compute(
                "ReduceScatter", OP.max,
                replica_groups=[list(range(NCORES))],
                ins=[cc_in.ap().opt()], outs=[cc_out.ap().opt()])

            # ---------- overlapped with collective ----------
            # psum_m = member_m + member_0 (pos pairing), in place
            for m in range(1, M):
                eng = nc.vector if m % 2 else nc.gpsimd
                eng.tensor_tensor(out=xmem[m], in0=xmem[m], in1=xmem[0],
                                  op=OP.add)
            # tsh = sum_m w8_m * dxm_m  (host-side gathered target logits)
            scr8 = smp.tile([128, M], F32, tag="scr8")
            tsh = smp.tile([128, 1], F32, tag="tsh")
            nc.vector.scalar_tensor_tensor(out=scr8, in0=dxm, scalar=1.0,
                                           in1=w8, op0=OP.mult, op1=OP.mult,
                                           accum_out=tsh)

            # ---------- combine + neg gather ----------
            # decode enc = gk mod 8192 in exact f32 (no mod/bitops on DVE):
            # e1 = gk/8192 (exact, pow2); q' = int(e1) (trunc or round);
            # e5 = (e1 - q')*8192 = enc or enc-8192; add 8192 if negative.
            gk = smp.tile([128, 1], F32, tag="gk")
            nc.scalar.dma_start(out=gk, in_=cc_out.ap())
            e1 = smp.tile([128, 1], F32, tag="e1")
            nc.vector.tensor_scalar(out=e1, in0=gk, scalar1=1.0 / 8192.0,
                                    scalar2=None, op0=OP.mult)
            e2i = smp.tile([128, 1], I32, tag="e2i")
            nc.vector.tensor_copy(out=e2i, in_=e1)
            e3 = smp.tile([128, 1], F32, tag="e3")
            nc.vector.tensor_copy(out=e3, in_=e2i)
            e5 = smp.tile([128, 1], F32, tag="e5")
            nc.vector.scalar_tensor_tensor(out=e5, in0=e1, scalar=1.0,
                                           in1=e3, op0=OP.mult,
                                           op1=OP.subtract)
            nc.vector.tensor_scalar(out=e5, in0=e5, scalar1=8192.0,
                                    scalar2=None, op0=OP.mult)
            corr = smp.tile([128, 1], F32, tag="corr")
            nc.vector.tensor_scalar(out=corr, in0=e5, scalar1=0.0,
                                    scalar2=8192.0, op0=OP.is_lt, op1=OP.mult)
            nc.vector.tensor_tensor(out=e5, in0=e5, in1=corr, op=OP.add)
            rowf = smp.tile([128, 1], F32, tag="rowf")
            nc.vector.tensor_scalar(out=rowf, in0=e5, scalar1=-1.0,
                                    scalar2=8191.0, op0=OP.mult, op1=OP.add)
            rowi = smp.tile([128, 1], I32, tag="rowi")
            nc.vector.tensor_copy(out=rowi, in_=rowf)
            negrow = pp.tile([128, C], F32, tag="negrow")
            for s in range(2):
                lo, hi = s * (C // 2), (s + 1) * (C // 2)
                nc.gpsimd.indirect_dma_start(
                    out=negrow[:, lo:hi], out_offset=None,
                    in_=x_d.ap(), element_offset=lo,
                    in_offset=bass.IndirectOffsetOnAxis(ap=rowi[:, 0:1],
                                                        axis=0))

            # ---------- class-sharded CE ----------
            # summed_m = psum_m + neg; exp+accum; Ln; weighted sum
            rs3 = []
            for m in range(1, M):
                eng = nc.vector if m % 2 else nc.gpsimd
                eng.tensor_tensor(out=xmem[m], in0=xmem[m], in1=negrow,
                                  op=OP.add)
            for m in range(1, M):
                dc = dcp.tile([128, C], F32, tag="dumpC")
                r3 = smp.tile([128, 1], F32, tag=f"rs3{m}")
                nc.scalar.activation(out=dc, in_=xmem[m], func=AF.Exp,
                                     bias=shC, scale=1.0, accum_out=r3)
                rs3.append(r3)
            # dneg = neg[p, class(p)] via eqmc diag extraction
            scrC = mp.tile([128, C], F32, tag="scrC")
            dneg = smp.tile([128, 1], F32, tag="dneg")
            nc.vector.scalar_tensor_tensor(out=scrC, in0=negrow, scalar=1.0,
                                           in1=eqmc, op0=OP.mult, op1=OP.mult,
                                           accum_out=dneg)
            lcat = smp.tile([128, M - 1], F32, tag="lcat")
            for m in range(1, M):
                nc.scalar.activation(out=lcat[:, m - 1:m], in_=rs3[m - 1],
                                     func=AF.Ln)
            scr7 = smp.tile([128, M - 1], F32, tag="scr7")
            wl = smp.tile([128, 1], F32, tag="wl")
            nc.vector.scalar_tensor_tensor(out=scr7, in0=lcat, scalar=1.0,
                                           in1=w7, op0=OP.mult, op1=OP.mult,
                                           accum_out=wl)
            # loss_p = wl + 8*SHIFT_C - tsh - 8*dneg
            a1 = smp.tile([128, 1], F32, tag="a1")
            nc.vector.tensor_scalar(out=a1, in0=dneg, scalar1=-8.0,
                                    scalar2=8.0 * SHIFT_C,
                                    op0=OP.mult, op1=OP.add)
            nc.vector.tensor_tensor(out=a1, in0=a1, in1=wl, op=OP.add)
            nc.vector.tensor_tensor(out=a1, in0=a1, in1=tsh, op=OP.subtract)

            pss = psb.tile([1, 1], F32, tag="psum_out")
            nc.tensor.matmul(pss, lhsT=a1, rhs=ones, start=True, stop=True)
            outt = smp.tile([1, 1], F32, tag="outt")
            nc.vector.tensor_copy(out=outt, in_=pss)
            nc.sync.dma_start(out=out_d.ap(), in_=outt)

    nc.compile()
    return nc


_NC_CACHE = {}


def get_nc():
    if "nc" not in _NC_CACHE:
        _NC_CACHE["nc"] = build_nc()
    return _NC_CACHE["nc"]


def make_in_maps(x, target):
    x = np.ascontiguousarray(np.asarray(x, dtype=np.float32))
    tgt = np.asarray(target).astype(np.int64)
    assert x.shape == (B, C) and tgt.shape == (B,)

    cid = np.arange(C, dtype=np.float32)
    cidb_full = np.ascontiguousarray(np.broadcast_to(cid, (128, C)))
    eye = np.eye(C, dtype=np.float32)

    # members[c] = sorted rows of class c (exactly M each)
    order = np.argsort(tgt, kind="stable")
    members = order.reshape(C, M).astype(np.int64)

    w8row = np.array([8.0, 2.0] + [1.0] * (M - 2), dtype=np.float32)
    w7row = np.array([2.0] + [1.0] * (M - 2), dtype=np.float32)
    w8_full = np.ascontiguousarray(np.broadcast_to(w8row, (128, M)))
    w7_full = np.ascontiguousarray(np.broadcast_to(w7row, (128, M - 1)))

    in_maps = []
    for k in range(NCORES):
        rows = slice(k * BL, (k + 1) * BL)
        tl = tgt[rows].astype(np.float32)
        gi = (k * BL + np.arange(BL)).astype(np.float32)
        ck = np.arange(k * CCL, (k + 1) * CCL)
        mem_k = members[ck]                      # [128, M]
        xmem = np.ascontiguousarray(
            x[mem_k.T.reshape(-1)])              # [M*128, C], m-major
        dxm = np.ascontiguousarray(
            x[mem_k, ck[:, None]].astype(np.float32))   # [128, M]
        in_maps.append({
            "x": x,
            "xloc": np.ascontiguousarray(x[rows]),
            "xmem": xmem,
            "cidb": cidb_full,
            "tcols": np.ascontiguousarray(tl.reshape(NT, 128).T),
            "encs": np.ascontiguousarray(
                (float(B) - 1.0 - gi).reshape(NT, 128).T),
            "eqmc": np.ascontiguousarray(eye[ck]),
            "dxm": dxm,
            "w8": w8_full,
            "w7": w7_full,
        })
    return in_maps


def kernel(x, target):
    nc = get_nc()
    in_maps = make_in_maps(x, target)
    res = run_bass_kernel_spmd(nc, in_maps, core_ids=list(range(NCORES)))
    total = sum(float(res.results[k]["partial"][0, 0]) for k in range(NCORES))
    return np.float32(total / B)


# revision 15
# speedup vs baseline: 2.1223x; 2.1223x over previous
"""Trainium2 Bass kernel for the hard-negative-mining set loss (v5).

Structure:
  * host: positives (first/second same-class occurrence) and per-class
    member tables are pure index bookkeeping on `target`; computed in
    numpy and shipped as gather tables (like the baseline's index tables).
  * device mining (row-sharded): per-class hardest negative via a packed
    f32 key K = 8192*q + enc - 2^24*same_mask, q = quantized -log prob,
    enc = 8191 - global_row. Quantization uses the fp32 "magic constant"
    rounding trick (adding 1.5*2^36 rounds to a multiple of 8192).
    partition_all_reduce collapses local rows; a 4KB ReduceScatter(max)
    yields each core's 128 owned classes' global keys directly.
  * device CE (class-sharded): per class only 7 distinct summed rows
    (anchors i0/i1 share one), members prefetched from t=0 via host
    tables; only the 128 neg rows (512KB) wait on the collective.
"""

import numpy as np

import concourse.bass as bass
import concourse.bacc as bacc
import concourse.tile as tile
from concourse import mybir
from concourse import bass_isa
from concourse.bass_utils import run_bass_kernel_spmd

B, C = 8192, 1024
NCORES = 8
BL = B // NCORES      # 1024 local mining rows per core
NT = BL // 128        # 8 row tiles
CCL = C // NCORES     # 128 classes owned per core
M = B // C            # 8 members per class

SHIFT_A = 10.0        # mining softmax shift
SHIFT_C = 14.0        # summed-logits softmax shift
QSCALE = 140.0        # log-prob quantization: 1/140 nat resolution
SCALE = QSCALE * 8192.0            # 1146880.0
M2 = 1.5 * (2.0 ** 36)             # magic: ulp(M2) = 8192
M2C = M2 + SCALE * SHIFT_A         # exact multiple of 8192
MASKC = -16777216.0                # -2^24 same-class exclusion
F32 = mybir.dt.float32
I32 = mybir.dt.int32
OP = mybir.AluOpType
AF = mybir.ActivationFunctionType


def build_nc():
    nc = bacc.Bacc("TRN2", target_bir_lowering=False, debug=False,
                   num_devices=NCORES)

    x_d = nc.dram_tensor("x", [B, C], F32, kind="ExternalInput")
    xloc_d = nc.dram_tensor("xloc", [BL, C], F32, kind="ExternalInput")
    xmem_d = nc.dram_tensor("xmem", [M * CCL, C], F32, kind="ExternalInput")
    combo_d = nc.dram_tensor("combo", [BL, C], F32, kind="ExternalInput")
    eqmc_d = nc.dram_tensor("eqmc", [128, C], F32, kind="ExternalInput")
    dxm_d = nc.dram_tensor("dxm", [128, M], F32, kind="ExternalInput")
    w8_d = nc.dram_tensor("w8", [128, M], F32, kind="ExternalInput")
    w7_d = nc.dram_tensor("w7", [128, M - 1], F32, kind="ExternalInput")
    out_d = nc.dram_tensor("partial", [1, 1], F32, kind="ExternalOutput")

    cc_in = nc.dram_tensor("cc_in", [1, C], F32)
    cc_out = nc.dram_tensor("cc_out", [1, C], F32)

    with tile.TileContext(nc) as tc:
        with (
            tc.tile_pool(name="persist", bufs=1) as pp,
            tc.tile_pool(name="rscr", bufs=3) as rp,
            tc.tile_pool(name="kscr", bufs=3) as kp,
            tc.tile_pool(name="mscr", bufs=3) as mp,
            tc.tile_pool(name="dumpC", bufs=2) as dcp,
            tc.tile_pool(name="small", bufs=6) as smp,
            tc.tile_pool(name="psB", bufs=1, space="PSUM") as psb,
        ):
            # ---------- input DMAs: xloc/combo interleaved (gate mining) ----
            xloc = []
            combo = []
            for t in range(NT):
                xt = pp.tile([128, C], F32, tag=f"xloc{t}")
                nc.sync.dma_start(out=xt, in_=xloc_d.ap()[t * 128:(t + 1) * 128, :])
                xloc.append(xt)
                cb = pp.tile([128, C], F32, tag=f"combo{t}")
                nc.sync.dma_start(out=cb, in_=combo_d.ap()[t * 128:(t + 1) * 128, :])
                combo.append(cb)
            dxm = pp.tile([128, M], F32, tag="dxm")
            nc.sync.dma_start(out=dxm, in_=dxm_d.ap())
            w8 = pp.tile([128, M], F32, tag="w8")
            nc.sync.dma_start(out=w8, in_=w8_d.ap())
            w7 = pp.tile([128, M - 1], F32, tag="w7")
            nc.sync.dma_start(out=w7, in_=w7_d.ap())
            xmem = []
            for m in range(M):
                xt = pp.tile([128, C], F32, tag=f"xmem{m}")
                nc.sync.dma_start(out=xt, in_=xmem_d.ap()[m * 128:(m + 1) * 128, :])
                xmem.append(xt)
            eqmc = pp.tile([128, C], F32, tag="eqmc")
            nc.sync.dma_start(out=eqmc, in_=eqmc_d.ap())

            ones = pp.tile([128, 1], F32, tag="ones")
            nc.vector.memset(ones, 1.0)
            shA = pp.tile([128, 1], F32, tag="shA")
            nc.vector.memset(shA, -SHIFT_A)
            shC = pp.tile([128, 1], F32, tag="shC")
            nc.vector.memset(shC, -SHIFT_C)

            # ---------- mining: packed-key build ----------
            # ACT queue: all Exp, all Ln, all Relu (one table load each)
            dumpA = pp.tile([128, C], F32, tag="dumpA")
            rs = []
            for t in range(NT):
                r = smp.tile([128, 1], F32, tag=f"rs{t}")
                nc.scalar.activation(out=dumpA, in_=xloc[t], func=AF.Exp,
                                     bias=shA, scale=1.0, accum_out=r)
                rs.append(r)
            lr = []
            for t in range(NT):
                l = smp.tile([128, 1], F32, tag=f"lr{t}")
                nc.scalar.activation(out=l, in_=rs[t], func=AF.Ln)
                lr.append(l)
            # b_t = f32(SCALE*lr + M2C): multiple of 8192 (carries lnrsum)
            bt = []
            for t in range(NT):
                b = smp.tile([128, 1], F32, tag=f"bt{t}")
                nc.vector.tensor_scalar(out=b, in0=lr[t], scalar1=SCALE,
                                        scalar2=M2C, op0=OP.mult, op1=OP.add)
                bt.append(b)
            # r_t = Relu(-SCALE*x + b_t) = b_t + 8192*q (fp32 rounds @8192)
            # K_t = (r_t - b_t) + combo_t  (combo = enc - 2^24*same_mask,
            # host-built; dual-op tensor_scalar measured ~19us/tile -- stt
            # is the fast dual-op path)
            Kacc = pp.tile([128, C], F32, tag="Kacc")
            for t in range(NT):
                rt = rp.tile([128, C], F32, tag="relu")
                nc.scalar.activation(out=rt, in_=xloc[t], func=AF.Relu,
                                     bias=bt[t], scale=-SCALE)
                if t == 0:
                    nc.vector.scalar_tensor_tensor(
                        out=Kacc, in0=rt, scalar=bt[t], op0=OP.subtract,
                        in1=combo[t], op1=OP.add)
                else:
                    kt = kp.tile([128, C], F32, tag="kt")
                    nc.vector.scalar_tensor_tensor(
                        out=kt, in0=rt, scalar=bt[t], op0=OP.subtract,
                        in1=combo[t], op1=OP.add)
                    nc.vector.tensor_tensor(out=Kacc, in0=Kacc, in1=kt,
                                            op=OP.max)

            # local per-class max across 128 partitions, then 4KB RS(max)
            Kpar = pp.tile([128, C], F32, tag="Kpar")
            nc.gpsimd.partition_all_reduce(out_ap=Kpar, in_ap=Kacc,
                                           channels=128,
                                           reduce_op=bass_isa.ReduceOp.max)
            nc.gpsimd.dma_start(out=cc_in.ap(), in_=Kpar[0:1, :])
            # AllToAll: my out chunk n = core n's local max for MY classes
            nc.gpsimd.collective_compute(
                "AllToAll", OP.bypass,
                replica_groups=[list(range(NCORES))],
                ins=[cc_in.ap().opt()], outs=[cc_out.ap().opt()])

            # ---------- overlapped with collective ----------
            # psum_m = member_m + member_0 (pos pairing), in place
            for m in range(1, M):
                eng = nc.vector if m % 2 else nc.gpsimd
                eng.tensor_tensor(out=xmem[m], in0=xmem[m], in1=xmem[0],
                                  op=OP.add)
            # tsh = sum_m w8_m * dxm_m  (host-side gathered target logits)
            scr8 = smp.tile([128, M], F32, tag="scr8")
            tsh = smp.tile([128, 1], F32, tag="tsh")
            nc.vector.scalar_tensor_tensor(out=scr8, in0=dxm, scalar=1.0,
                                           in1=w8, op0=OP.mult, op1=OP.mult,
                                           accum_out=tsh)

            # ---------- combine + neg gather ----------
            # decode enc = gk mod 8192 in exact f32 (no mod/bitops on DVE):
            # e1 = gk/8192 (exact, pow2); q' = int(e1) (trunc or round);
            # e5 = (e1 - q')*8192 = enc or enc-8192; add 8192 if negative.
            k8 = smp.tile([NCORES, CCL], F32, tag="k8")
            k8src = bass.AP(tensor=cc_out.ap().tensor, offset=0,
                            ap=[[CCL, NCORES], [1, CCL]])
            nc.scalar.dma_start(out=k8, in_=k8src)
            k8r = smp.tile([NCORES, CCL], F32, tag="k8r")
            nc.gpsimd.partition_all_reduce(out_ap=k8r, in_ap=k8,
                                           channels=NCORES,
                                           reduce_op=bass_isa.ReduceOp.max)
            gk = smp.tile([128, 1], F32, tag="gk")
            nc.scalar.dma_start(out=gk, in_=k8r[0:1, 0:CCL])
            e1 = smp.tile([128, 1], F32, tag="e1")
            nc.vector.tensor_scalar(out=e1, in0=gk, scalar1=1.0 / 8192.0,
                                    scalar2=None, op0=OP.mult)
            e2i = smp.tile([128, 1], I32, tag="e2i")
            nc.vector.tensor_copy(out=e2i, in_=e1)
            e3 = smp.tile([128, 1], F32, tag="e3")
            nc.vector.tensor_copy(out=e3, in_=e2i)
            e5 = smp.tile([128, 1], F32, tag="e5")
            nc.vector.scalar_tensor_tensor(out=e5, in0=e1, scalar=1.0,
                                           in1=e3, op0=OP.mult,
                                           op1=OP.subtract)
            nc.vector.tensor_scalar(out=e5, in0=e5, scalar1=8192.0,
                                    scalar2=None, op0=OP.mult)
            corr = smp.tile([128, 1], F32, tag="corr")
            nc.vector.tensor_scalar(out=corr, in0=e5, scalar1=0.0,
                                    scalar2=8192.0, op0=OP.is_lt, op1=OP.mult)
            nc.vector.tensor_tensor(out=e5, in0=e5, in1=corr, op=OP.add)
            rowf = smp.tile([128, 1], F32, tag="rowf")
            nc.vector.tensor_scalar(out=rowf, in0=e5, scalar1=-1.0,
                                    scalar2=8191.0, op0=OP.mult, op1=OP.add)
            rowi = smp.tile([128, 1], I32, tag="rowi")
            nc.vector.tensor_copy(out=rowi, in_=rowf)
            negrow = pp.tile([128, C], F32, tag="negrow")
            for s in range(2):
                lo, hi = s * (C // 2), (s + 1) * (C // 2)
                nc.gpsimd.indirect_dma_start(
                    out=negrow[:, lo:hi], out_offset=None,
                    in_=x_d.ap(), element_offset=lo,
                    in_offset=bass.IndirectOffsetOnAxis(ap=rowi[:, 0:1],
                                                        axis=0))

            # ---------- class-sharded CE ----------
            # summed_m = psum_m + neg; exp+accum; Ln; weighted sum
            rs3 = []
            for m in range(1, M):
                eng = nc.vector if m % 2 else nc.gpsimd
                eng.tensor_tensor(out=xmem[m], in0=xmem[m], in1=negrow,
                                  op=OP.add)
            for m in range(1, M):
                dc = dcp.tile([128, C], F32, tag="dumpC")
                r3 = smp.tile([128, 1], F32, tag=f"rs3{m}")
                nc.scalar.activation(out=dc, in_=xmem[m], func=AF.Exp,
                                     bias=shC, scale=1.0, accum_out=r3)
                rs3.append(r3)
            # dneg = neg[p, class(p)] via eqmc diag extraction
            scrC = mp.tile([128, C], F32, tag="scrC")
            dneg = smp.tile([128, 1], F32, tag="dneg")
            nc.vector.scalar_tensor_tensor(out=scrC, in0=negrow, scalar=1.0,
                                           in1=eqmc, op0=OP.mult, op1=OP.mult,
                                           accum_out=dneg)
            lcat = smp.tile([128, M - 1], F32, tag="lcat")
            for m in range(1, M):
                nc.scalar.activation(out=lcat[:, m - 1:m], in_=rs3[m - 1],
                                     func=AF.Ln)
            scr7 = smp.tile([128, M - 1], F32, tag="scr7")
            wl = smp.tile([128, 1], F32, tag="wl")
            nc.vector.scalar_tensor_tensor(out=scr7, in0=lcat, scalar=1.0,
                                           in1=w7, op0=OP.mult, op1=OP.mult,
                                           accum_out=wl)
            # loss_p = wl + 8*SHIFT_C - tsh - 8*dneg
            a1 = smp.tile([128, 1], F32, tag="a1")
            nc.vector.tensor_scalar(out=a1, in0=dneg, scalar1=-8.0,
                                    scalar2=8.0 * SHIFT_C,
                                    op0=OP.mult, op1=OP.add)
            nc.vector.tensor_tensor(out=a1, in0=a1, in1=wl, op=OP.add)
            nc.vector.tensor_tensor(out=a1, in0=a1, in1=tsh, op=OP.subtract)

            pss = psb.tile([1, 1], F32, tag="psum_out")
            nc.tensor.matmul(pss, lhsT=a1, rhs=ones, start=True, stop=True)
            outt = smp.tile([1, 1], F32, tag="outt")
            nc.vector.tensor_copy(out=outt, in_=pss)
            nc.sync.dma_start(out=out_d.ap(), in_=outt)

    nc.compile()
    return nc


_NC_CACHE = {}


def get_nc():
    if "nc" not in _NC_CACHE:
        _NC_CACHE["nc"] = build_nc()
    return _NC_CACHE["nc"]


def make_in_maps(x, target):
    x = np.ascontiguousarray(np.asarray(x, dtype=np.float32))
    tgt = np.asarray(target).astype(np.int64)
    assert x.shape == (B, C) and tgt.shape == (B,)

    eye = np.eye(C, dtype=np.float32)

    # members[c] = sorted rows of class c (exactly M each)
    order = np.argsort(tgt, kind="stable")
    members = order.reshape(C, M).astype(np.int64)

    w8row = np.array([8.0, 2.0] + [1.0] * (M - 2), dtype=np.float32)
    w7row = np.array([2.0] + [1.0] * (M - 2), dtype=np.float32)
    w8_full = np.ascontiguousarray(np.broadcast_to(w8row, (128, M)))
    w7_full = np.ascontiguousarray(np.broadcast_to(w7row, (128, M - 1)))

    in_maps = []
    for k in range(NCORES):
        rows = slice(k * BL, (k + 1) * BL)
        tloc = tgt[rows]
        gi = k * BL + np.arange(BL)
        ck = np.arange(k * CCL, (k + 1) * CCL)
        mem_k = members[ck]                      # [128, M]
        xmem = np.ascontiguousarray(
            x[mem_k.T.reshape(-1)])              # [M*128, C], m-major
        dxm = np.ascontiguousarray(
            x[mem_k, ck[:, None]].astype(np.float32))   # [128, M]
        # combo[j, c] = (B-1 - global_row_j) - 2^24 * (target_j == c)
        combo = np.repeat((float(B) - 1.0 - gi).astype(np.float32)[:, None],
                          C, axis=1)
        combo[np.arange(BL), tloc] += MASKC
        in_maps.append({
            "x": x,
            "xloc": np.ascontiguousarray(x[rows]),
            "xmem": xmem,
            "combo": np.ascontiguousarray(combo),
            "eqmc": np.ascontiguousarray(eye[ck]),
            "dxm": dxm,
            "w8": w8_full,
            "w7": w7_full,
        })
    return in_maps


def kernel(x, target):
    nc = get_nc()
    in_maps = make_in_maps(x, target)
    res = run_bass_kernel_spmd(nc, in_maps, core_ids=list(range(NCORES)))
    total = sum(float(res.results[k]["partial"][0, 0]) for k in range(NCORES))
    return np.float32(total / B)


# revision 20
# speedup vs baseline: 2.2608x; 1.0653x over previous
"""Trainium2 Bass kernel for the hard-negative-mining set loss (v5).

Structure:
  * host: positives (first/second same-class occurrence) and per-class
    member tables are pure index bookkeeping on `target`; computed in
    numpy and shipped as gather tables (like the baseline's index tables).
  * device mining (row-sharded): per-class hardest negative via a packed
    f32 key K = 8192*q + enc - 2^24*same_mask, q = quantized -log prob,
    enc = 8191 - global_row. Quantization uses the fp32 "magic constant"
    rounding trick (adding 1.5*2^36 rounds to a multiple of 8192).
    partition_all_reduce collapses local rows; a 4KB ReduceScatter(max)
    yields each core's 128 owned classes' global keys directly.
  * device CE (class-sharded): per class only 7 distinct summed rows
    (anchors i0/i1 share one), members prefetched from t=0 via host
    tables; only the 128 neg rows (512KB) wait on the collective.
"""

import numpy as np

import concourse.bass as bass
import concourse.bacc as bacc
import concourse.tile as tile
from concourse import mybir
from concourse import bass_isa
from concourse.bass_utils import run_bass_kernel_spmd

B, C = 8192, 1024
NCORES = 8
BL = B // NCORES      # 1024 local mining rows per core
NT = BL // 128        # 8 row tiles
CCL = C // NCORES     # 128 classes owned per core
M = B // C            # 8 members per class

SHIFT_A = 10.0        # mining softmax shift
SHIFT_C = 14.0        # summed-logits softmax shift
QSCALE = 140.0        # log-prob quantization: 1/140 nat resolution
SCALE = QSCALE * 8192.0            # 1146880.0
M2 = 1.5 * (2.0 ** 36)             # magic: ulp(M2) = 8192
M2C = M2 + SCALE * SHIFT_A         # exact multiple of 8192
MASKC = -16777216.0                # -2^24 same-class exclusion
F32 = mybir.dt.float32
I32 = mybir.dt.int32
OP = mybir.AluOpType
AF = mybir.ActivationFunctionType


def build_nc():
    nc = bacc.Bacc("TRN2", target_bir_lowering=False, debug=False,
                   num_devices=NCORES)

    x_d = nc.dram_tensor("x", [B, C], F32, kind="ExternalInput")
    xloc_d = nc.dram_tensor("xloc", [BL, C], F32, kind="ExternalInput")
    xmem_d = nc.dram_tensor("xmem", [M * CCL, C], F32, kind="ExternalInput")
    combo_d = nc.dram_tensor("combo", [BL, C], F32, kind="ExternalInput")
    eqmc_d = nc.dram_tensor("eqmc", [128, C], F32, kind="ExternalInput")
    dxm_d = nc.dram_tensor("dxm", [128, M], F32, kind="ExternalInput")
    w8_d = nc.dram_tensor("w8", [128, M], F32, kind="ExternalInput")
    w7_d = nc.dram_tensor("w7", [128, M - 1], F32, kind="ExternalInput")
    out_d = nc.dram_tensor("partial", [1, 1], F32, kind="ExternalOutput")

    cc_in = nc.dram_tensor("cc_in", [1, C], F32)
    cc_out = nc.dram_tensor("cc_out", [1, C], F32)

    with tile.TileContext(nc) as tc:
        with (
            tc.tile_pool(name="persist", bufs=1) as pp,
            tc.tile_pool(name="rscr", bufs=3) as rp,
            tc.tile_pool(name="kscr", bufs=3) as kp,
            tc.tile_pool(name="mscr", bufs=3) as mp,
            tc.tile_pool(name="dumpC", bufs=2) as dcp,
            tc.tile_pool(name="small", bufs=6) as smp,
            tc.tile_pool(name="psB", bufs=1, space="PSUM") as psb,
        ):
            # ---------- input DMAs ----------
            # mining inputs from gpsimd (its startup fence clears ~8us
            # before sync's); xloc tiles first -- they gate the ACT chain
            xloc = []
            combo = []
            for t in range(NT):
                xt = pp.tile([128, C], F32, tag=f"xloc{t}")
                nc.gpsimd.dma_start(out=xt, in_=xloc_d.ap()[t * 128:(t + 1) * 128, :])
                xloc.append(xt)
            for t in range(NT):
                cb = pp.tile([128, C], F32, tag=f"combo{t}")
                nc.gpsimd.dma_start(out=cb, in_=combo_d.ap()[t * 128:(t + 1) * 128, :])
                combo.append(cb)
            dxm = pp.tile([128, M], F32, tag="dxm")
            nc.sync.dma_start(out=dxm, in_=dxm_d.ap())
            w8 = pp.tile([128, M], F32, tag="w8")
            nc.sync.dma_start(out=w8, in_=w8_d.ap())
            w7 = pp.tile([128, M - 1], F32, tag="w7")
            nc.sync.dma_start(out=w7, in_=w7_d.ap())
            xmem = []
            for m in range(M):
                xt = pp.tile([128, C], F32, tag=f"xmem{m}")
                nc.sync.dma_start(out=xt, in_=xmem_d.ap()[m * 128:(m + 1) * 128, :])
                xmem.append(xt)
            eqmc = pp.tile([128, C], F32, tag="eqmc")
            nc.sync.dma_start(out=eqmc, in_=eqmc_d.ap())

            ones = pp.tile([128, 1], F32, tag="ones")
            nc.vector.memset(ones, 1.0)
            shA = pp.tile([128, 1], F32, tag="shA")
            nc.vector.memset(shA, -SHIFT_A)
            shC = pp.tile([128, 1], F32, tag="shC")
            nc.vector.memset(shC, -SHIFT_C)

            # ---------- mining: packed-key build ----------
            # single Ln/bt over [128,NT] accumulators: keeps the ACT queue
            # at 2 table loads (Exp, Ln) regardless of scheduler order
            dumpA = pp.tile([128, C], F32, tag="dumpA")
            rscat = smp.tile([128, NT], F32, tag="rscat")
            for t in range(NT):
                nc.scalar.activation(out=dumpA, in_=xloc[t], func=AF.Exp,
                                     bias=shA, scale=1.0,
                                     accum_out=rscat[:, t:t + 1])
            lrcat = smp.tile([128, NT], F32, tag="lrcat")
            nc.scalar.activation(out=lrcat, in_=rscat, func=AF.Ln)
            # b_t = f32(SCALE*lr + M2C): multiple of 8192 (carries lnrsum)
            btcat = smp.tile([128, NT], F32, tag="btcat")
            nc.vector.tensor_scalar(out=btcat, in0=lrcat, scalar1=SCALE,
                                    scalar2=M2C, op0=OP.mult, op1=OP.add)
            bt = [btcat[:, t:t + 1] for t in range(NT)]
            # r_t = Relu(-SCALE*x + b_t) = b_t + 8192*q (fp32 rounds @8192)
            # K_t = (r_t - b_t) + combo_t  (combo = enc - 2^24*same_mask,
            # host-built; dual-op tensor_scalar measured ~19us/tile -- stt
            # is the fast dual-op path)
            Kacc = pp.tile([128, C], F32, tag="Kacc")
            for t in range(NT):
                rt = rp.tile([128, C], F32, tag="relu")
                nc.scalar.activation(out=rt, in_=xloc[t], func=AF.Relu,
                                     bias=bt[t], scale=-SCALE)
                if t == 0:
                    nc.vector.scalar_tensor_tensor(
                        out=Kacc, in0=rt, scalar=bt[t], op0=OP.subtract,
                        in1=combo[t], op1=OP.add)
                else:
                    kt = kp.tile([128, C], F32, tag="kt")
                    nc.vector.scalar_tensor_tensor(
                        out=kt, in0=rt, scalar=bt[t], op0=OP.subtract,
                        in1=combo[t], op1=OP.add)
                    # stt (1.28us) beats tensor_tensor (3us) on DVE
                    nc.vector.scalar_tensor_tensor(
                        out=Kacc, in0=Kacc, scalar=0.0, op0=OP.add,
                        in1=kt, op1=OP.max)

            # local per-class max across 128 partitions, then 4KB RS(max)
            Kpar = pp.tile([128, C], F32, tag="Kpar")
            nc.gpsimd.partition_all_reduce(out_ap=Kpar, in_ap=Kacc,
                                           channels=128,
                                           reduce_op=bass_isa.ReduceOp.max)
            nc.gpsimd.dma_start(out=cc_in.ap(), in_=Kpar[0:1, :])
            # AllToAll: my out chunk n = core n's local max for MY classes
            nc.gpsimd.collective_compute(
                "AllToAll", OP.bypass,
                replica_groups=[list(range(NCORES))],
                ins=[cc_in.ap().opt()], outs=[cc_out.ap().opt()])

            # ---------- overlapped with collective ----------
            # psum_m = member_m + member_0 (pos pairing), in place
            for m in range(1, M):
                if m % 2:
                    nc.vector.scalar_tensor_tensor(
                        out=xmem[m], in0=xmem[m], scalar=0.0, op0=OP.add,
                        in1=xmem[0], op1=OP.add)
                else:
                    nc.gpsimd.tensor_tensor(out=xmem[m], in0=xmem[m],
                                            in1=xmem[0], op=OP.add)
            # tsh = sum_m w8_m * dxm_m  (host-side gathered target logits)
            scr8 = smp.tile([128, M], F32, tag="scr8")
            tsh = smp.tile([128, 1], F32, tag="tsh")
            nc.vector.scalar_tensor_tensor(out=scr8, in0=dxm, scalar=1.0,
                                           in1=w8, op0=OP.mult, op1=OP.mult,
                                           accum_out=tsh)

            # ---------- combine + neg gather ----------
            # decode enc = gk mod 8192 in exact f32 (no mod/bitops on DVE):
            # e1 = gk/8192 (exact, pow2); q' = int(e1) (trunc or round);
            # e5 = (e1 - q')*8192 = enc or enc-8192; add 8192 if negative.
            k8 = smp.tile([NCORES, CCL], F32, tag="k8")
            k8src = bass.AP(tensor=cc_out.ap().tensor, offset=0,
                            ap=[[CCL, NCORES], [1, CCL]])
            nc.scalar.dma_start(out=k8, in_=k8src)
            k8r = smp.tile([NCORES, CCL], F32, tag="k8r")
            nc.gpsimd.partition_all_reduce(out_ap=k8r, in_ap=k8,
                                           channels=NCORES,
                                           reduce_op=bass_isa.ReduceOp.max)
            gk = smp.tile([128, 1], F32, tag="gk")
            nc.scalar.dma_start(out=gk, in_=k8r[0:1, 0:CCL])
            e1 = smp.tile([128, 1], F32, tag="e1")
            nc.vector.tensor_scalar(out=e1, in0=gk, scalar1=1.0 / 8192.0,
                                    scalar2=None, op0=OP.mult)
            e2i = smp.tile([128, 1], I32, tag="e2i")
            nc.vector.tensor_copy(out=e2i, in_=e1)
            e3 = smp.tile([128, 1], F32, tag="e3")
            nc.vector.tensor_copy(out=e3, in_=e2i)
            e5 = smp.tile([128, 1], F32, tag="e5")
            nc.vector.scalar_tensor_tensor(out=e5, in0=e1, scalar=1.0,
                                           in1=e3, op0=OP.mult,
                                           op1=OP.subtract)
            nc.vector.tensor_scalar(out=e5, in0=e5, scalar1=8192.0,
                                    scalar2=None, op0=OP.mult)
            corr = smp.tile([128, 1], F32, tag="corr")
            nc.vector.tensor_scalar(out=corr, in0=e5, scalar1=0.0,
                                    scalar2=8192.0, op0=OP.is_lt, op1=OP.mult)
            nc.vector.tensor_tensor(out=e5, in0=e5, in1=corr, op=OP.add)
            rowf = smp.tile([128, 1], F32, tag="rowf")
            nc.vector.tensor_scalar(out=rowf, in0=e5, scalar1=-1.0,
                                    scalar2=8191.0, op0=OP.mult, op1=OP.add)
            rowi = smp.tile([128, 1], I32, tag="rowi")
            nc.vector.tensor_copy(out=rowi, in_=rowf)
            negrow = pp.tile([128, C], F32, tag="negrow")
            for s in range(2):
                lo, hi = s * (C // 2), (s + 1) * (C // 2)
                nc.gpsimd.indirect_dma_start(
                    out=negrow[:, lo:hi], out_offset=None,
                    in_=x_d.ap(), element_offset=lo,
                    in_offset=bass.IndirectOffsetOnAxis(ap=rowi[:, 0:1],
                                                        axis=0))

            # ---------- class-sharded CE ----------
            # summed_m = psum_m + neg (DVE stt, pipelined with ACT exps)
            rscat3 = smp.tile([128, M - 1], F32, tag="rscat3")
            for m in range(1, M):
                nc.vector.scalar_tensor_tensor(
                    out=xmem[m], in0=xmem[m], scalar=0.0, op0=OP.add,
                    in1=negrow, op1=OP.add)
                dc = dcp.tile([128, C], F32, tag="dumpC")
                nc.scalar.activation(out=dc, in_=xmem[m], func=AF.Exp,
                                     bias=shC, scale=1.0,
                                     accum_out=rscat3[:, m - 1:m])
            # dneg = neg[p, class(p)] via eqmc diag extraction
            scrC = mp.tile([128, C], F32, tag="scrC")
            dneg = smp.tile([128, 1], F32, tag="dneg")
            nc.vector.scalar_tensor_tensor(out=scrC, in0=negrow, scalar=1.0,
                                           in1=eqmc, op0=OP.mult, op1=OP.mult,
                                           accum_out=dneg)
            lcat = smp.tile([128, M - 1], F32, tag="lcat")
            nc.scalar.activation(out=lcat, in_=rscat3, func=AF.Ln)
            scr7 = smp.tile([128, M - 1], F32, tag="scr7")
            wl = smp.tile([128, 1], F32, tag="wl")
            nc.vector.scalar_tensor_tensor(out=scr7, in0=lcat, scalar=1.0,
                                           in1=w7, op0=OP.mult, op1=OP.mult,
                                           accum_out=wl)
            # loss_p = wl + 8*SHIFT_C - tsh - 8*dneg
            a1 = smp.tile([128, 1], F32, tag="a1")
            nc.vector.tensor_scalar(out=a1, in0=dneg, scalar1=-8.0,
                                    scalar2=8.0 * SHIFT_C,
                                    op0=OP.mult, op1=OP.add)
            nc.vector.tensor_tensor(out=a1, in0=a1, in1=wl, op=OP.add)
            nc.vector.tensor_tensor(out=a1, in0=a1, in1=tsh, op=OP.subtract)

            pss = psb.tile([1, 1], F32, tag="psum_out")
            nc.tensor.matmul(pss, lhsT=a1, rhs=ones, start=True, stop=True)
            outt = smp.tile([1, 1], F32, tag="outt")
            nc.vector.tensor_copy(out=outt, in_=pss)
            nc.sync.dma_start(out=out_d.ap(), in_=outt)

    nc.compile()
    return nc


_NC_CACHE = {}


def get_nc():
    if "nc" not in _NC_CACHE:
        _NC_CACHE["nc"] = build_nc()
    return _NC_CACHE["nc"]


def make_in_maps(x, target):
    x = np.ascontiguousarray(np.asarray(x, dtype=np.float32))
    tgt = np.asarray(target).astype(np.int64)
    assert x.shape == (B, C) and tgt.shape == (B,)

    eye = np.eye(C, dtype=np.float32)

    # members[c] = sorted rows of class c (exactly M each)
    order = np.argsort(tgt, kind="stable")
    members = order.reshape(C, M).astype(np.int64)

    w8row = np.array([8.0, 2.0] + [1.0] * (M - 2), dtype=np.float32)
    w7row = np.array([2.0] + [1.0] * (M - 2), dtype=np.float32)
    w8_full = np.ascontiguousarray(np.broadcast_to(w8row, (128, M)))
    w7_full = np.ascontiguousarray(np.broadcast_to(w7row, (128, M - 1)))

    in_maps = []
    for k in range(NCORES):
        rows = slice(k * BL, (k + 1) * BL)
        tloc = tgt[rows]
        gi = k * BL + np.arange(BL)
        ck = np.arange(k * CCL, (k + 1) * CCL)
        mem_k = members[ck]                      # [128, M]
        xmem = np.ascontiguousarray(
            x[mem_k.T.reshape(-1)])              # [M*128, C], m-major
        dxm = np.ascontiguousarray(
            x[mem_k, ck[:, None]].astype(np.float32))   # [128, M]
        # combo[j, c] = (B-1 - global_row_j) - 2^24 * (target_j == c)
        combo = np.repeat((float(B) - 1.0 - gi).astype(np.float32)[:, None],
                          C, axis=1)
        combo[np.arange(BL), tloc] += MASKC
        in_maps.append({
            "x": x,
            "xloc": np.ascontiguousarray(x[rows]),
            "xmem": xmem,
            "combo": np.ascontiguousarray(combo),
            "eqmc": np.ascontiguousarray(eye[ck]),
            "dxm": dxm,
            "w8": w8_full,
            "w7": w7_full,
        })
    return in_maps


def kernel(x, target):
    nc = get_nc()
    in_maps = make_in_maps(x, target)
    res = run_bass_kernel_spmd(nc, in_maps, core_ids=list(range(NCORES)))
    total = sum(float(res.results[k]["partial"][0, 0]) for k in range(NCORES))
    return np.float32(total / B)
